# revision 81
# baseline (speedup 1.0000x reference)
"""ARMA-style GNN message passing on 8 TRN2 NeuronCores.

Reference computation (per layer, 7 layers):
    m   = h @ W                                  [N, CH]
    agg = segment_sum(w[:,None] * m[dst], src)   [N, CH]
    h'  = relu(agg + h @ V + b)
then logits = h @ Wd + bd.

Strategy (graph/data parallel over nodes, ReduceScatter aggregation):
  - 8 cores own 1250 nodes each (padded to 1280 = 10 blocks of 128).
  - Edge (s, d) is processed by the core owning d (the message SOURCE row):
    core q computes m for its own nodes only, keeps it local, and produces
    PARTIAL aggregates for ALL 80 global src node-blocks from its own edges.
    A ReduceScatter(add) then sums the 8 partials and hands each core the
    aggregate rows for its own nodes.  Output of the collective is only
    [1280, 512] bf16 per core (vs a 10.5 MB AllGather), and all message
    gathers are core-local.
  - Per global src block b, the host dedups the edges' dst rows (unique
    rows only), builds gather index chunks of 128 rows and [128 x 128]
    bf16 "C" matrices with the summed degree weights; segment-sum becomes
    C.T @ gathered_rows on PE, accumulated in PSUM over the block's chunks.
  - Gathers are batched (8 chunks per dma_gather) to amortize the SWDGE
    fixed overhead; the collective is split 6/4 over the node blocks so the
    first ReduceScatter overlaps the second half's messages and the h'
    phase of the first 6 blocks hides under the second collective.
  - h@V (+bias) for the last 4 blocks is precomputed into SBUF before the
    second collective lands, shortening the post-collective critical chain
    to add + relu + PE-transpose + next-layer m.
  - Per-layer W/V weights are streamed through a double buffer during the
    collective window; the final dense layer is fused into the last h'
    phase per block; host concatenates the per-core outputs.

All matmuls run in bf16 with fp32 PSUM accumulation.
"""
import numpy as np
import ml_dtypes

import concourse.bass as bass
import concourse.tile as tile
import concourse.mybir as mybir
import bass_rust as _bass_rust
from concourse.vector_clock import ScopedClock
from concourse.bass_utils import run_bass_kernel_spmd
from concourse.masks import make_identity
from concourse import library_config

# ---------------------------------------------------------------- constants
N_NODES = 10000
N_EDGES = 160000
IN_F = 256
CH = 512
N_LABELS = 1440
NCORES = 8
NPC = N_NODES // NCORES      # 1250 nodes per core
P = 128
NBL = 10                     # node blocks per core (10*128 = 1280)
NPAD = NBL * P               # padded nodes per core
GBL = NCORES * NBL           # 80 global src node blocks
NLAYERS = 7
KG1 = IN_F // P              # 2 contraction blocks in layer 1
KGC = CH // P                # 4 contraction blocks in layers 2..7
FIN_CHUNK = 480              # 1440 = 3 * 480, fits one PSUM bank in f32
GBATCH = 8                   # gather chunks batched per dma_gather call

BF = mybir.dt.bfloat16
F32 = mybir.dt.float32
BFNP = ml_dtypes.bfloat16

# processing order of global src blocks: j-major so that each core's
# low-j blocks complete first (enables split collectives)
GB_ORDER = [p * NBL + j for j in range(NBL) for p in range(NCORES)]


# ------------------------------------------------------- walrus workarounds
def _patched_drain_and_barrier(self, tick_clock, wait_clock):
    # This walrus build rejects >1-2 sync waits on one TPB_CTRL; put the
    # kernel-tail drain's waits on separate preceding SP nops instead.
    nc = self.nc
    probe = nc.sync.nop(nofuse=True, hint="drain_waits")
    wait_clock.add_sem_waits(probe.ins, ScopedClock({None: tick_clock.global_clock}))
    si = probe.ins.sync_info
    waits = list(si.on_wait) if si is not None else []
    if len(waits) > 1:
        si.on_wait = waits[:1]
        for i in range(1, len(waits)):
            n2 = nc.sync.nop(nofuse=True, hint=f"drain_waits_{i}")
            n2.ins.sync_info = mybir.SyncInfo(on_wait=[waits[i]], on_update=[])
    nc.sync.drain()
    nc.all_engine_barrier()
    assert self.sems is not None
    popped = nc._tile_sem_poison_stack.pop()
    assert popped is self._sem_poison
    nc.clear_and_free_semaphores(list(self.sems.allocated().values()))
    nc.all_engine_barrier()


tile.TileContext._drain_and_barrier = _patched_drain_and_barrier


def _split_excess_waits(nc, limit=1):
    # Same ISA restriction for ordinary instructions: hoist excess sync
    # waits onto injected same-engine nops placed just before.
    for func in nc.m.functions:
        for bb in func.blocks:
            out = []
            for ins in bb.instructions:
                si = ins.sync_info
                if si is not None and si.on_wait and len(si.on_wait) > limit:
                    waits = list(si.on_wait)
                    excess, keep = waits[:-limit], waits[-limit:]
                    for i in range(0, len(excess), limit):
                        out.append(mybir.InstNoOp(
                            name=f"{ins.name}_xw{i}",
                            engine=ins.engine,
                            ins=[], outs=[],
                            sync_info=mybir.SyncInfo(
                                on_wait=excess[i:i + limit], on_update=[]),
                        ))
                    si.on_wait = keep
                out.append(ins)
            bb.instructions[:] = out


# ------------------------------------------------------------- host prep
def _prep_edges(src, dst):
    """Partition edges by owning core of dst; per (core, global src block)
    dedup dst rows and build gather-index chunks + C matrices (summed
    degree weights).  Returns (kcb, idx_tabs, c_tabs): kcb[i] = chunk count
    of the i-th block in GB_ORDER (same on all cores)."""
    src = np.asarray(src).astype(np.int64)
    dst = np.asarray(dst).astype(np.int64)
    deg_out = np.maximum(np.bincount(src, minlength=N_NODES), 1.0).astype(np.float64)
    deg_in = np.maximum(np.bincount(dst, minlength=N_NODES), 1.0).astype(np.float64)
    w = (1.0 / np.sqrt(deg_out[src] * deg_in[dst])).astype(np.float32)

    core = dst // NPC                       # owner of the message source row
    ldst = dst - core * NPC                 # local gather row
    sloc = src % NPC
    gb = (src // NPC) * NBL + sloc // P     # global src block
    slot = sloc % P                         # column within the block

    # bucket edges by (core, gb)
    order = np.lexsort((gb, core))
    core_s, gb_s = core[order], gb[order]
    ldst_s, slot_s, w_s = ldst[order], slot[order], w[order]
    counts = np.zeros((NCORES, GBL), np.int64)
    np.add.at(counts, (core_s, gb_s), 1)
    starts = np.zeros((NCORES, GBL), np.int64)
    flat = counts.ravel().cumsum()
    starts.ravel()[1:] = flat[:-1]

    # unique-dst rows per (core, gb)
    uniq = [[None] * GBL for _ in range(NCORES)]
    nuniq = np.zeros((NCORES, GBL), np.int64)
    inv_all = [[None] * GBL for _ in range(NCORES)]
    for q in range(NCORES):
        for b in range(GBL):
            s0, cnt = starts[q, b], counts[q, b]
            u, inv = np.unique(ldst_s[s0:s0 + cnt], return_inverse=True)
            uniq[q][b] = u
            inv_all[q][b] = inv
            nuniq[q, b] = len(u)

    kcb = [max(1, int(-(-nuniq[:, b].max() // P))) for b in GB_ORDER]
    neb = sum(kcb)

    idx_tabs, c_tabs = [], []
    for q in range(NCORES):
        # chunk c's 128 gather rows live at int16 columns c*8..c*8+7:
        # within-call index i = (c-c0)*128 + p  ->  [i%16, i//16]
        # = [p%16, (c-c0)*8 + p//16]  (replicated across the 8 stripes)
        idx_t = np.zeros((P, neb * (P // 16)), np.int16)
        c_t = np.zeros((P, neb * P), np.float32)
        col = 0
        for oi, b in enumerate(GB_ORDER):
            s0, cnt = starts[q, b], counts[q, b]
            u = uniq[q][b]
            inv = inv_all[q][b]          # edge -> position in u
            sl = slot_s[s0:s0 + cnt]
            ww = w_s[s0:s0 + cnt]
            for k in range(kcb[oi]):
                lo, hi = k * P, min((k + 1) * P, len(u))
                if hi > lo:
                    rows = np.zeros(P, np.int16)
                    rows[:hi - lo] = u[lo:hi]
                    pp = np.arange(P)
                    for s in range(P // 16):
                        idx_t[s * 16 + pp % 16, col * (P // 16) + pp // 16] = rows
                    sel = (inv >= lo) & (inv < hi)
                    np.add.at(c_t, (inv[sel] - lo, col * P + sl[sel]), ww[sel])
                col += 1
        idx_tabs.append(idx_t)
        c_tabs.append(c_t.astype(BFNP))
    return kcb, idx_tabs, c_tabs


def _pack_lhsT(xT, kg):
    """[kg*128, NPAD] -> [128, kg*NPAD] (partition-major kg blocks)."""
    return np.ascontiguousarray(
        xT.reshape(kg, P, NPAD).transpose(1, 0, 2).reshape(P, kg * NPAD))


def _pack_rhs(Wm, kg, n):
    """[kg*128, n] -> [128, kg*n]."""
    return np.ascontiguousarray(
        Wm.reshape(kg, P, n).transpose(1, 0, 2).reshape(P, kg * n))


# ------------------------------------------------------------- device build
def _build(kcb, repeat=1):
    neb = sum(kcb)
    nc = bass.Bass("TRN2", target_bir_lowering=False, debug=False,
                   num_devices=NCORES,
                   num_swdge_queues=4)

    def din(name, shape, dt):
        return nc.dram_tensor(name, shape, dt, kind="ExternalInput").ap()

    xT = din("xT", [P, KG1 * NPAD], BF)
    idx = din("idx", [P, neb * (P // 16)], mybir.dt.int16)
    ctab = din("ctab", [P, neb * P], BF)
    w1 = din("w1", [P, KG1 * CH], BF)
    v1 = din("v1", [P, KG1 * CH], BF)
    wk = din("wk", [P, 6 * KGC * CH], BF)
    vk = din("vk", [P, 6 * KGC * CH], BF)
    wd = din("wd", [P, KGC * N_LABELS], BF)
    ball = din("ball", [P, NLAYERS * CH], BF)
    bdr = din("bdr", [P, N_LABELS], BF)
    out = nc.dram_tensor("out", [NPAD, N_LABELS], F32, kind="ExternalOutput").ap()

    # chunk -> ordered-block mapping
    chunk_block = []          # index into GB_ORDER position for each chunk col
    for oi in range(GBL):
        chunk_block.extend([oi] * kcb[oi])

    with tile.TileContext(nc) as tc:
        with (
            tc.tile_pool(name="const", bufs=1) as cp,
            tc.tile_pool(name="ht", bufs=2) as htp,
            tc.tile_pool(name="mout", bufs=2) as mp,
            tc.tile_pool(name="msg", bufs=4) as msgp,
            tc.tile_pool(name="aggf", bufs=2) as aggfp,
            tc.tile_pool(name="hact", bufs=2) as hp,
            tc.tile_pool(name="wv", bufs=2) as wvp,
            tc.tile_pool(name="outs", bufs=1) as op,
            tc.tile_pool(name="psm", bufs=2, space="PSUM") as psm,
            tc.tile_pool(name="psagg", bufs=4, space="PSUM") as psagg,
            tc.tile_pool(name="pstr", bufs=2, space="PSUM") as pstr,
            tc.tile_pool(name="dram", bufs=1, space="DRAM") as dram,
        ):
            # ---- constants needed for layer 0 first
            nc.gpsimd.load_library(library_config.mlp)
            # uneven node-block split: the first collective covers HA of
            # the 10 blocks so more of the h' phase hides under the second
            HA = 6
            HALVES = [(0, HA), (HA, (NBL - HA) // 2),
                      (HA + (NBL - HA) // 2, NBL - HA - (NBL - HA) // 2)]
            chalf = sum(kcb[:HA * NCORES])
            widths = {GBATCH}
            for n in (chalf, neb - chalf):
                if n % GBATCH:
                    widths.add(n % GBATCH)
            # shared num_idxs registers (one per distinct batch width)
            nidx_regs = {}
            for cw in widths:
                nidx_regs[cw] = nc.gpsimd.to_reg(cw * P)
            w1_t = cp.tile([P, KG1 * CH], BF)
            nc.sync.dma_start(w1_t[:], w1[:])
            v1_t = cp.tile([P, KG1 * CH], BF)
            nc.sync.dma_start(v1_t[:], v1[:])
            idx_t = cp.tile([P, neb * (P // 16)], mybir.dt.int16)
            nc.sync.dma_start(idx_t[:], idx[:])
            ctab_t = cp.tile([P, neb * P], BF)
            ball_t = cp.tile([P, NLAYERS * CH], BF)
            nc.sync.dma_start(ball_t[:], ball[:])
            ident = cp.tile([P, P], BF)
            make_identity(nc, ident[:])
            # late constants (first needed at the final layer)
            wd_t = cp.tile([P, KGC * N_LABELS], BF)
            bdr_t = cp.tile([P, N_LABELS], BF)

            def emit_m_block(b, h0, lhsT_t, wt, kg, m_sb):
                """m_b = h_b @ W -> bf16 into column b-h0 of m_sb."""
                m_ps = psm.tile([P, CH], F32, tag="m")
                for g in range(kg):
                    nc.tensor.matmul(
                        m_ps[:],
                        lhsT_t[:, g * NPAD + b * P:g * NPAD + (b + 1) * P],
                        wt[:, g * CH:(g + 1) * CH],
                        start=(g == 0), stop=(g == kg - 1))
                j = b - h0
                nc.scalar.activation(
                    m_sb[:, j * CH:(j + 1) * CH], m_ps[:],
                    mybir.ActivationFunctionType.Copy)

            def emit_m_flush(h0, hw, m_dram, m_sb):
                """one DMA: hw blocks of m -> m_dram[h0:h0+hw]."""
                nc.sync.dma_start(
                    m_dram[h0:h0 + hw].rearrange("j p e -> p j e"),
                    m_sb[:, :hw * CH].rearrange("p (j e) -> p j e", e=CH))

            for rep in range(repeat):
                hT_cur = None
                wv_cur = None
                for l in range(NLAYERS):
                    kg = KG1 if l == 0 else KGC
                    if l == 0:
                        hT_cur = htp.tile([P, KGC * NPAD], BF, tag="hT",
                                          name="hT_x")
                        lhsT_t = hT_cur
                        nc.sync.dma_start(hT_cur[:, :KG1 * NPAD], xT[:])
                        wt = w1_t[:, :]
                        vt = v1_t[:, :]
                    else:
                        lhsT_t = hT_cur
                        wt = wv_cur[:, :KGC * CH]
                        vt = wv_cur[:, KGC * CH:]

                    if l == 0:
                        # standalone M phase for layer 0 (merged into the
                        # previous layer's tail for l > 0)
                        m_dram = dram.tile([NBL, P, CH], BF, tag=f"m{l & 1}")
                        for h0, hw in HALVES:
                            m_sb = mp.tile([P, HA * CH], BF, tag="msb")
                            for b in range(h0, h0 + hw):
                                emit_m_block(b, h0, lhsT_t, wt, kg, m_sb)
                            emit_m_flush(h0, hw, m_dram, m_sb)
                        if rep == 0:
                            # bulky tables load behind x/W1 so layer-0 m
                            # starts immediately; done before aggs need them
                            nc.sync.dma_start(ctab_t[:, :chalf * P],
                                              ctab[:, :chalf * P])
                            nc.sync.dma_start(ctab_t[:, chalf * P:],
                                              ctab[:, chalf * P:])
                            nc.sync.dma_start(wd_t[:], wd[:])
                            nc.sync.dma_start(bdr_t[:], bdr[:])
                    else:
                        m_dram = m_dram_next

                    # --- gathers + partial aggregates + collectives.
                    # Pool-queue order: [gathers A, first B batch, RS-A,
                    # remaining B gathers, RS-B] — RS-A dispatches as soon
                    # as half A's rows are written without stalling half B's
                    # gather pipeline behind it.
                    mflat = m_dram[:].rearrange("j p e -> (j p) e")
                    msg_tiles = {}

                    def emit_gathers(clo, chi):
                        for c0 in range(clo, chi, GBATCH):
                            cw = min(GBATCH, chi - c0)
                            mt = msgp.tile([P, GBATCH * CH], BF, tag="msg",
                                           name=f"msg{c0}")
                            nc.gpsimd.dma_gather(
                                mt[:].rearrange(
                                    "p (k e) -> p k e", e=CH)[:, :cw, :],
                                mflat,
                                idx_t[:, c0 * (P // 16):(c0 + cw) * (P // 16)],
                                cw * P, nidx_regs[cw], CH)
                            msg_tiles[c0] = mt

                    def emit_aggs(h0, hw, rs_in, col):
                        clo = chalf if h0 >= HA else 0
                        for jj in range(hw):
                            j = h0 + jj
                            agg_row = aggfp.tile([P, NCORES * CH], BF,
                                                 tag="ar", name=f"ar{j}")
                            for pp in range(NCORES):
                                oi = j * NCORES + pp
                                agg_ps = psagg.tile([P, CH], F32, tag="agg",
                                                    name=f"agg{oi}")
                                for k in range(kcb[oi]):
                                    mt = msg_tiles[
                                        clo + ((col - clo) // GBATCH) * GBATCH]
                                    jc = (col - clo) % GBATCH
                                    nc.tensor.matmul(
                                        agg_ps[:],
                                        ctab_t[:, col * P:(col + 1) * P],
                                        mt[:, jc * CH:(jc + 1) * CH],
                                        start=(k == 0),
                                        stop=(k == kcb[oi] - 1))
                                    col += 1
                                if pp % 2 == 0:
                                    nc.vector.tensor_copy(
                                        agg_row[:, pp * CH:(pp + 1) * CH],
                                        agg_ps[:])
                                else:
                                    nc.scalar.activation(
                                        agg_row[:, pp * CH:(pp + 1) * CH],
                                        agg_ps[:],
                                        mybir.ActivationFunctionType.Copy)
                            nc.sync.dma_start(
                                rs_in[:, jj].rearrange("c p e -> p c e"),
                                agg_row[:].rearrange("p (c e) -> p c e", e=CH))
                        return col

                    def emit_rs(half, hw, rs_in):
                        rs_out = dram.tile([hw, P, CH], BF,
                                           tag=f"rs_out{l}_{half}",
                                           name=f"rs_out{l}_{half}")
                        nc.gpsimd.collective_compute(
                            "ReduceScatter", mybir.AluOpType.add,
                            replica_groups=[list(range(NCORES))],
                            ins=[rs_in[:].opt()], outs=[rs_out[:].opt()])
                        return rs_out

                    HB1 = (NBL - HA) // 2        # 2 + 2 tail segments
                    HB2 = NBL - HA - HB1
                    emit_gathers(0, chalf)
                    rs_in0 = dram.tile([NCORES, HA, P, CH], BF, tag="rs_in0")
                    col = emit_aggs(0, HA, rs_in0, 0)
                    bsplit = min(chalf + GBATCH, neb)
                    emit_gathers(chalf, bsplit)
                    rs_outs = [emit_rs(0, HA, rs_in0)]
                    emit_gathers(bsplit, neb)
                    rs_in1 = dram.tile([NCORES, HB1, P, CH], BF,
                                       tag="rs_in1")
                    col = emit_aggs(HA, HB1, rs_in1, col)
                    rs_outs.append(emit_rs(1, HB1, rs_in1))
                    rs_in2 = dram.tile([NCORES, HB2, P, CH], BF,
                                       tag="rs_in2")
                    emit_aggs(HA + HB1, HB2, rs_in2, col)
                    rs_outs.append(emit_rs(2, HB2, rs_in2))
                    if l < NLAYERS - 1:
                        # stream next layer's W/V in the collective window
                        wv_next = wvp.tile([P, 2 * KGC * CH], BF, tag="wv")
                        nc.sync.dma_start(
                            wv_next[:, :KGC * CH],
                            wk[:, l * KGC * CH:(l + 1) * KGC * CH])
                        nc.sync.dma_start(
                            wv_next[:, KGC * CH:],
                            vk[:, l * KGC * CH:(l + 1) * KGC * CH])

                    # precompute h@V + b for the second-half blocks in the
                    # pre-collective idle window: shortens the post-RS-B
                    # critical chain to add+relu+transpose+m
                    hvb = hp.tile([P, (NBL - HA) * CH], BF, tag="hvb",
                                  bufs=1)
                    for bb in range(NBL - HA):
                        b = HA + bb
                        hv_ps = psm.tile([P, CH], F32, tag="m",
                                         name=f"hv{b}")
                        for g in range(kg):
                            nc.tensor.matmul(
                                hv_ps[:],
                                lhsT_t[:, g * NPAD + b * P:
                                       g * NPAD + (b + 1) * P],
                                vt[:, g * CH:(g + 1) * CH],
                                start=(g == 0), stop=(g == kg - 1))
                        nc.scalar.activation(
                            hvb[:, bb * CH:(bb + 1) * CH], hv_ps[:],
                            mybir.ActivationFunctionType.Copy)
                        nc.vector.tensor_add(
                            hvb[:, bb * CH:(bb + 1) * CH],
                            hvb[:, bb * CH:(bb + 1) * CH],
                            ball_t[:, l * CH:(l + 1) * CH])

                    # --- h' = relu(agg + h@V + b); transpose; next-layer m
                    last = (l == NLAYERS - 1)
                    if not last:
                        hT_next = htp.tile([P, KGC * NPAD], BF, tag="hT")
                        m_dram_next = dram.tile([NBL, P, CH], BF,
                                                tag=f"m{(l + 1) & 1}")
                        wt_next = wv_next[:, :KGC * CH]
                    for half, (h0, hw) in enumerate(HALVES):
                        agg_all = hp.tile([P, HA * CH], BF, tag="aggsb")
                        nc.sync.dma_start(
                            agg_all[:, :hw * CH].rearrange(
                                "p (j e) -> p j e", e=CH),
                            rs_outs[half][:].rearrange("j p e -> p j e"))
                        if not last:
                            m_sb = mp.tile([P, HA * CH], BF, tag="msb")
                        for bb in range(hw):
                            b = h0 + bb
                            if h0 >= HA:
                                # fast path: hv+b precomputed; bf16 add+relu
                                h_pre = hp.tile([P, CH], BF, tag="h",
                                                name=f"hpre{b}")
                                nc.vector.tensor_add(
                                    h_pre[:],
                                    agg_all[:, bb * CH:(bb + 1) * CH],
                                    hvb[:, (b - HA) * CH:(b - HA + 1) * CH])
                                h_bf = hp.tile([P, CH], BF, tag="h")
                                nc.scalar.activation(
                                    h_bf[:], h_pre[:],
                                    mybir.ActivationFunctionType.Relu)
                            else:
                                h_ps = psm.tile([P, CH], F32, tag="m")
                                nc.tensor.matmul(
                                    h_ps[:], ident[:],
                                    agg_all[:, bb * CH:(bb + 1) * CH],
                                    start=True, stop=False)
                                for g in range(kg):
                                    nc.tensor.matmul(
                                        h_ps[:],
                                        lhsT_t[:, g * NPAD + b * P:
                                               g * NPAD + (b + 1) * P],
                                        vt[:, g * CH:(g + 1) * CH],
                                        start=False, stop=(g == kg - 1))
                                nc.vector.tensor_add(
                                    h_ps[:], h_ps[:],
                                    ball_t[:, l * CH:(l + 1) * CH])
                                h_bf = hp.tile([P, CH], BF, tag="h")
                                nc.scalar.activation(
                                    h_bf[:], h_ps[:],
                                    mybir.ActivationFunctionType.Relu)
                            # 4 transposes into one PSUM tile, one copy out
                            tr_ps = pstr.tile([P, KGC * P], BF, tag="tr")
                            for cg in range(KGC):
                                nc.tensor.transpose(
                                    tr_ps[:, cg * P:(cg + 1) * P],
                                    h_bf[:, cg * P:(cg + 1) * P], ident[:])
                            if not last:
                                hT_view = hT_next[:].rearrange(
                                    "p (g n) -> p g n", n=NPAD)
                                nc.vector.tensor_copy(
                                    hT_view[:, :, b * P:(b + 1) * P],
                                    tr_ps[:].rearrange(
                                        "p (g n) -> p g n", n=P))
                                # next layer's m for this block
                                emit_m_block(b, h0, hT_next, wt_next, KGC,
                                             m_sb)
                            else:
                                # final dense for this block
                                hT_b = hp.tile([P, KGC * P], BF, tag="hTb")
                                nc.scalar.activation(
                                    hT_b[:], tr_ps[:],
                                    mybir.ActivationFunctionType.Copy)
                                o_sb = op.tile([P, N_LABELS], F32, tag="o")
                                for c in range(3):
                                    fin_ps = psagg.tile([P, CH], F32,
                                                        tag="agg")
                                    for g in range(KGC):
                                        nc.tensor.matmul(
                                            fin_ps[:, :FIN_CHUNK],
                                            hT_b[:, g * P:(g + 1) * P],
                                            wd_t[:, g * N_LABELS + c * FIN_CHUNK:
                                                 g * N_LABELS + (c + 1) * FIN_CHUNK],
                                            start=(g == 0), stop=(g == KGC - 1))
                                    nc.vector.tensor_add(
                                        fin_ps[:, :FIN_CHUNK],
                                        fin_ps[:, :FIN_CHUNK],
                                        bdr_t[:, c * FIN_CHUNK:(c + 1) * FIN_CHUNK])
                                    nc.scalar.activation(
                                        o_sb[:, c * FIN_CHUNK:(c + 1) * FIN_CHUNK],
                                        fin_ps[:, :FIN_CHUNK],
                                        mybir.ActivationFunctionType.Copy)
                                if rep == repeat - 1:
                                    nc.sync.dma_start(
                                        out[b * P:(b + 1) * P, :], o_sb[:])
                        if not last:
                            emit_m_flush(h0, hw, m_dram_next, m_sb)
                    if not last:
                        hT_cur = hT_next
                        wv_cur = wv_next

    _split_excess_waits(nc)
    # lower extension instructions (dma_gather): insert GPSIMD library
    # loads and populate .instr bytes (else walrus fails "ISA wrong length")
    mask = {}
    for lib in library_config.all_libraries:
        for it in lib.instructions:
            mask[it] = mask.get(it, 0) | (1 << lib.index)
    _bass_rust.insert_library_loads(
        nc, mask, len(library_config.all_libraries), library_config.standard.index)
    mybir.codegen_inst_isa_subclasses(nc)
    return nc


# ------------------------------------------------------------- entry point
def kernel(x, src, dst, W1, V1, b1, Wk, Vk, bk, Wd, bd, _repeat=1, _nc_cache={}):
    x = np.asarray(x, np.float32)
    kcb, idx_tabs, c_tabs = _prep_edges(src, dst)

    key = (tuple(kcb), _repeat)
    if key not in _nc_cache:
        _nc_cache[key] = _build(kcb, repeat=_repeat)
    nc = _nc_cache[key]

    # weights (replicated, host-packed)
    w1p = _pack_rhs(np.asarray(W1, np.float32), KG1, CH).astype(BFNP)
    v1p = _pack_rhs(np.asarray(V1, np.float32), KG1, CH).astype(BFNP)
    wkp = np.concatenate(
        [_pack_rhs(np.asarray(Wk[i], np.float32), KGC, CH) for i in range(6)],
        axis=1).astype(BFNP)
    vkp = np.concatenate(
        [_pack_rhs(np.asarray(Vk[i], np.float32), KGC, CH) for i in range(6)],
        axis=1).astype(BFNP)
    wdp = _pack_rhs(np.asarray(Wd, np.float32), KGC, N_LABELS).astype(BFNP)
    ballv = np.concatenate(
        [np.asarray(b1, np.float32)] + [np.asarray(bk[i], np.float32)
                                        for i in range(6)])
    ballp = np.broadcast_to(ballv, (P, NLAYERS * CH)).astype(BFNP)
    bdp = np.broadcast_to(np.asarray(bd, np.float32),
                          (P, N_LABELS)).astype(BFNP)

    in_maps = []
    for p in range(NCORES):
        xp = np.zeros((NPAD, IN_F), np.float32)
        xp[:NPC] = x[p * NPC:(p + 1) * NPC]
        xTp = _pack_lhsT(np.ascontiguousarray(xp.T), KG1).astype(BFNP)
        in_maps.append({
            "xT": xTp, "idx": idx_tabs[p], "ctab": c_tabs[p],
            "w1": w1p, "v1": v1p, "wk": wkp, "vk": vkp, "wd": wdp,
            "ball": ballp, "bdr": bdp,
        })

    res = run_bass_kernel_spmd(nc, in_maps, core_ids=list(range(NCORES)))
    outp = np.empty((N_NODES, N_LABELS), np.float32)
    for p in range(NCORES):
        outp[p * NPC:(p + 1) * NPC] = res.results[p]["out"][:NPC]
    return outp
